# revision 1
# baseline (speedup 1.0000x reference)
"""GCN message passing on 8 Trainium2 NeuronCores (Bass/Tile SPMD).

out = segment_sum(feature[src], dst, N=50000) @ W.T + b

Distribution (per the sharding hint): dst-nodes and their incoming edges are
partitioned across the 8 cores (6250 nodes each). Each core receives only its
feature shard; the full table is replicated on-device via AllGather (halo
exchange). Per-core, src rows are fetched with SWDGE dma_gather (two table
halves to fit int16 indices), segment-summed over 128-node windows with
one-hot matmuls accumulating in PSUM (race-free), and the replicated 128x128
linear + bias is fused into the same pass.

Host-side work is limited to input staging: graph preprocessing (edge
grouping by (core, window, src-half)) depends only on src/dst and is cached
across calls on a checksum; dense inputs are cast to fp16 per call.

Self-contained: requires only numpy + the concourse/jax runtime available in
the environment. Falls back to a pure-numpy path if the device path is
unavailable.
"""

import zlib
import numpy as np

N_NODES = 50000
D = 128
N_CORES = 8
NS = N_NODES // N_CORES          # 6250 nodes per core
WINDOWS = (NS + 127) // 128      # 49
CAP = 1024                       # gather slots per (window, half)
T_HALF = CAP // 128              # 8 msg tiles per half
TT = 2 * T_HALF                  # 16 msg tiles per window
HALF = N_NODES // 2              # int16 gather index split
N_BUFS = 4

_STATE = {}


# --------------------------------------------------------------------------
# device program
# --------------------------------------------------------------------------

def _build_nc():
    import concourse.bacc as bacc
    import concourse.mybir as mybir
    from concourse import tile

    f16, f32, i16, u32 = (mybir.dt.float16, mybir.dt.float32,
                          mybir.dt.int16, mybir.dt.uint32)
    W, T, NB = WINDOWS, T_HALF, N_BUFS
    nc = bacc.Bacc(None, target_bir_lowering=False, num_swdge_queues=4)

    feat = nc.dram_tensor("feat", [NS, D], f16, kind="ExternalInput")
    gidx_d = nc.dram_tensor("gidx", [128, 2 * W, CAP // 16], i16,
                            kind="ExternalInput")
    slots_d = nc.dram_tensor("slots", [128, W, TT], f16, kind="ExternalInput")
    cnts_d = nc.dram_tensor("cnts", [1, 2 * W], u32, kind="ExternalInput")
    wt_d = nc.dram_tensor("wt", [D, D], f16, kind="ExternalInput")
    b_d = nc.dram_tensor("brep", [128, D], f32, kind="ExternalInput")
    reps_d = nc.dram_tensor("reps", [1, 1], u32, kind="ExternalInput")
    out_d = nc.dram_tensor("out", [NS, D], f32, kind="ExternalOutput")

    cc_in = nc.dram_tensor("cc_in", [NS, D], f16)
    # Two half-tables (separate tensors => precise RAW deps): table h holds
    # every rank's half-shard h, concatenated by rank. Gathers of half 0 only
    # wait on the first AllGather; the second overlaps with them.
    tables = [nc.dram_tensor(f"table{h}", [HALF, D], f16, addr_space="Shared")
              for h in (0, 1)]
    half_aps = [tables[0][:, :], tables[1][:, :]]
    HS = NS // 2                     # 3125 rows per half-shard

    with tile.TileContext(nc) as tc:
        with (
            tc.tile_pool(name="const", bufs=1) as cpool,
            tc.tile_pool(name="msgs", bufs=1) as mpool,
            tc.tile_pool(name="oneh", bufs=1) as opool,
            tc.tile_pool(name="work", bufs=1) as wpool,
            tc.tile_pool(name="pacc", bufs=1, space="PSUM") as pacc,
            tc.tile_pool(name="pout", bufs=1, space="PSUM") as pout,
        ):
            # halo exchange: replicate the feature table on every core,
            # split into two AllGathers so gathers can start after the first
            nc.sync.dma_start(out=cc_in[:], in_=feat[:])
            for h in (0, 1):
                nc.gpsimd.collective_compute(
                    "AllGather", mybir.AluOpType.bypass,
                    replica_groups=[list(range(N_CORES))],
                    ins=[cc_in[h * HS:(h + 1) * HS, :]], outs=[tables[h][:]])

            gidx = cpool.tile([128, 2 * W, CAP // 16], i16)
            slots = cpool.tile([128, W, TT], f16)
            cnts = cpool.tile([1, 2 * W], u32)
            wt_s = cpool.tile([D, D], f16)
            b_s = cpool.tile([128, D], f32)
            iota = cpool.tile([128, TT, 128], f16)
            nc.sync.dma_start(out=gidx[:], in_=gidx_d[:])
            nc.sync.dma_start(out=slots[:], in_=slots_d[:])
            nc.sync.dma_start(out=cnts[:], in_=cnts_d[:])
            nc.sync.dma_start(out=wt_s[:], in_=wt_d[:])
            nc.sync.dma_start(out=b_s[:], in_=b_d[:])
            nc.gpsimd.iota(iota[:], pattern=[[0, TT], [1, 128]],
                           channel_multiplier=0,
                           allow_small_or_imprecise_dtypes=True)

            msgs_bufs = [mpool.tile([128, TT, D], f16, tag=f"m{i}",
                                    name=f"m{i}") for i in range(NB)]
            oneh_bufs = [opool.tile([128, TT, 128], f16, tag=f"o{i}",
                                    name=f"o{i}") for i in range(NB)]
            aggT_bufs = [wpool.tile([128, 128], f16, tag=f"a{i}",
                                    name=f"a{i}") for i in range(3)]
            outs_bufs = [wpool.tile([128, D], f32, tag=f"u{i}",
                                    name=f"u{i}") for i in range(3)]
            pacc_bufs = [pacc.tile([128, 128], f32, tag=f"pa{i}",
                                   name=f"pa{i}") for i in range(NB)]
            pout_bufs = [pout.tile([128, D], f32, tag=f"po{i}",
                                   name=f"po{i}") for i in range(2)]
            for t_ in msgs_bufs:
                nc.gpsimd.memset(t_[:], 0.0)   # no NaN bits reach the PE
            # 4 rotating count registers, loaded 4 ops ahead so the loads
            # hide inside the SWDGE ring waits instead of extending the chain
            cnt_regs = [nc.gpsimd.alloc_register(f"cnt_reg{j}")
                        for j in range(4)]
            dummy = cpool.tile([1, 1], f32, name="dummy")
            nc.gpsimd.memset(dummy[:], 0.0)
            NOPS = 2 * W

            def body(_iv=None):
                nc.scalar.mul(dummy[:], dummy[:], 1.0)  # For_i: all engines
                for j in range(4):
                    nc.gpsimd.reg_load(cnt_regs[j], cnts[0:1, j:j + 1])
                for w in range(W):
                    msgs = msgs_bufs[w % NB]
                    oneh = oneh_bufs[w % NB]
                    ps = pacc_bufs[w % NB]
                    aggT = aggT_bufs[w % 3]
                    outs = outs_bufs[w % 3]
                    po = pout_bufs[w % 2]
                    for h in (0, 1):
                        k = 2 * w + h
                        nc.gpsimd.dma_gather(
                            msgs[:, h * T:(h + 1) * T, :], half_aps[h],
                            gidx[:, k, :], CAP, cnt_regs[k % 4], D,
                            queue_num=k % 4)
                        if k + 4 < NOPS:
                            nc.gpsimd.reg_load(cnt_regs[k % 4],
                                               cnts[0:1, k + 4:k + 5])
                    nc.vector.tensor_tensor(
                        oneh[:, :, :], iota[:, :, :],
                        slots[:, w, :].broadcast_to([128, TT, 128]),
                        mybir.AluOpType.is_equal)
                    for t in range(TT):
                        nc.tensor.matmul(ps[:, :], msgs[:, t, :],
                                         oneh[:, t, :],
                                         start=(t == 0), stop=(t == TT - 1))
                    nc.vector.tensor_copy(aggT[:, :], ps[:, :])
                    nc.tensor.matmul(po[:, :], aggT[:, :], wt_s[:, :])
                    nc.vector.tensor_tensor(outs[:, :], po[:, :], b_s[:, :],
                                            mybir.AluOpType.add)
                    rows = min(128, NS - w * 128)
                    nc.sync.dma_start(out=out_d[w * 128:w * 128 + rows, :],
                                      in_=outs[:rows, :])

            reps_t = cpool.tile([1, 1], u32)
            nc.sync.dma_start(out=reps_t[:], in_=reps_d[:])
            regs = nc.alloc_registers("reps_regs")
            for rh in regs.handles:
                nc.engines[rh.engine].reg_load(rh, reps_t[0:1, 0:1])
            reps = nc.snap(regs, donate=True, min_val=1, max_val=1 << 20)
            with tc.For_i(0, reps) as _i:
                body(_i)
    nc.compile()
    return nc


class _Runner:
    """Cached PJRT runner (jit/shard_map built once; device-cached inputs)."""

    def __init__(self, nc):
        import jax
        import concourse.mybir as mybir
        from jax.sharding import Mesh, PartitionSpec, NamedSharding
        from jax.experimental.shard_map import shard_map
        from concourse import bass2jax
        from concourse.bass2jax import _bass_exec_p, partition_id_tensor

        bass2jax.install_neuronx_cc_hook()
        self.nc = nc
        in_names, out_names, out_avals = [], [], []
        pname = nc.partition_id_tensor.name if nc.partition_id_tensor else None
        for alloc in nc.m.functions[0].allocations:
            if not isinstance(alloc, mybir.MemoryLocationSet):
                continue
            name = alloc.memorylocations[0].name
            if alloc.kind == "ExternalInput":
                if name != pname:
                    in_names.append(name)
            elif alloc.kind == "ExternalOutput":
                out_names.append(name)
                out_avals.append(jax.core.ShapedArray(
                    tuple(alloc.tensor_shape), mybir.dt.np(alloc.dtype)))
        self.in_names = in_names
        self.out_names = out_names
        all_in = list(in_names) + ([pname] if pname else [])

        def _body(*args):
            operands = list(args)
            if pname is not None:
                operands.append(partition_id_tensor())
            return tuple(_bass_exec_p.bind(
                *operands, out_avals=tuple(out_avals),
                in_names=tuple(all_in), out_names=tuple(out_names),
                lowering_input_output_aliases=(),
                sim_require_finite=True, sim_require_nnan=True, nc=nc))

        devices = jax.devices()[:N_CORES]
        mesh = Mesh(np.asarray(devices), ("core",))
        self.sharding = NamedSharding(mesh, PartitionSpec("core"))
        self.jitted = jax.jit(shard_map(
            _body, mesh=mesh,
            in_specs=(PartitionSpec("core"),) * len(in_names),
            out_specs=(PartitionSpec("core"),) * len(out_names),
            check_rep=False))
        self._put = lambda a: jax.device_put(a, self.sharding)

    def put(self, arr):
        return self._put(arr)

    def __call__(self, gin):
        outs = self.jitted(*[gin[n] for n in self.in_names])
        return {n: np.asarray(o) for n, o in zip(self.out_names, outs)}


# --------------------------------------------------------------------------
# host-side preprocessing
# --------------------------------------------------------------------------

def _prep_graph(src, dst):
    """Group edges by (core, window, src-half); graph-only, cacheable."""
    src = np.asarray(src).astype(np.int64, copy=False)
    dst = np.asarray(dst).astype(np.int64, copy=False)
    E = src.shape[0]
    W = WINDOWS

    core = dst // NS
    dloc = dst - core * NS
    w = dloc >> 7
    slot = dloc & 127
    # permuted half-table index: src row (c, o) lives in table h=(o>=HS) at
    # row c*HS + (o % HS), matching the rank-concat layout of AllGather h
    HS = NS // 2
    sc = src // NS
    so = src - sc * NS
    h = (so >= HS).astype(np.int64)
    g16 = (sc * HS + so - h * HS).astype(np.int16)

    group = (core * W + w) * 2 + h
    NG = N_CORES * W * 2
    order = np.argsort(group, kind="stable")
    gs = group[order]
    counts = np.bincount(gs, minlength=NG)
    if counts.max() > CAP:
        raise OverflowError("window/half capacity exceeded")
    starts = np.zeros(NG, np.int64)
    np.cumsum(counts[:-1], out=starts[1:])
    pos = np.arange(E) - starts[gs]

    garr = np.full((NG, CAP), -1, np.int16)
    garr[gs, pos] = g16[order]
    sarr = np.full((NG, CAP), -1, np.float16)
    sarr[gs, pos] = slot[order].astype(np.float16)

    cnts = np.maximum(counts, 1).astype(np.uint32)
    garr[counts == 0, 0] = 0

    gp = garr.reshape(N_CORES, 2 * W, CAP // 16, 16).transpose(0, 3, 1, 2)
    gp = np.broadcast_to(gp[:, None], (N_CORES, 8, 16, 2 * W, CAP // 16))
    gidx = np.ascontiguousarray(gp).reshape(N_CORES * 128, 2 * W, CAP // 16)

    sv = sarr.reshape(N_CORES, W, 2, T_HALF, 128).transpose(0, 4, 1, 2, 3)
    slots_a = np.ascontiguousarray(sv).reshape(N_CORES * 128, W, TT)

    return {
        "gidx": gidx.astype(np.int16),
        "slots": slots_a.astype(np.float16),
        "cnts": np.ascontiguousarray(cnts.reshape(N_CORES, 2 * W)),
    }


def _graph_key(src, dst):
    s = np.ascontiguousarray(src)
    d = np.ascontiguousarray(dst)
    return (s.shape[0], zlib.adler32(s.tobytes()), zlib.adler32(d.tobytes()))


def _get_state():
    if "runner" not in _STATE:
        _STATE["runner"] = _Runner(_build_nc())
    return _STATE


def _kernel_device(feature, src, dst, W, b):
    st = _get_state()
    runner = st["runner"]

    key = _graph_key(src, dst)
    if st.get("graph_key") != key:
        g = _prep_graph(src, dst)
        st["graph"] = {k: runner.put(v) for k, v in g.items()}
        st["graph_key"] = key
        st["reps1"] = runner.put(
            np.tile(np.array([[1]], np.uint32), (N_CORES, 1)))

    feat16 = np.ascontiguousarray(feature, dtype=np.float16)
    wt = np.ascontiguousarray(np.asarray(W).T, dtype=np.float16)
    brep = np.broadcast_to(np.asarray(b, np.float32)[None, :], (128, D))
    gin = {
        **st["graph"],
        "feat": feat16,
        "wt": np.ascontiguousarray(np.tile(wt, (N_CORES, 1))),
        "brep": np.ascontiguousarray(np.tile(brep, (N_CORES, 1))),
        "reps": st["reps1"],
    }
    out = runner(gin)["out"]
    # stash device-resident inputs for the test harness's timing runs
    st["last_gin"] = {k: (v if k in ("gidx", "slots", "cnts", "reps")
                          else runner.put(v)) for k, v in gin.items()}
    return np.ascontiguousarray(out.reshape(N_NODES, D))


def _kernel_numpy(feature, src, dst, W, b):
    """Host fallback (correct for any shapes)."""
    feature = np.asarray(feature, dtype=np.float32)
    n = feature.shape[0]
    agg = np.zeros_like(feature)
    np.add.at(agg, np.asarray(dst).astype(np.int64),
              feature[np.asarray(src).astype(np.int64)])
    return agg @ np.asarray(W, np.float32).T + np.asarray(b, np.float32)


def kernel(feature, src, dst, W, b):
    feature = np.asarray(feature)
    if (feature.shape != (N_NODES, D) or np.asarray(W).shape != (D, D)):
        return _kernel_numpy(feature, src, dst, W, b)
    try:
        return _kernel_device(feature, src, dst, W, b)
    except Exception:
        return _kernel_numpy(feature, src, dst, W, b)

